# revision 5
# baseline (speedup 1.0000x reference)
"""Multi-head attention (B=4, S=2048, D=1024, H=16) on 8 Trainium2 NeuronCores.

Sharding: tensor-parallel over heads. Core c owns heads 2c, 2c+1 (a 128-wide
slice of the model dim). Each core computes Q/K/V projections for its head
slice over all tokens, causal attention for its 2 heads, and a partial output
projection (contraction over its 128 x-dims). The host sums the 8 partial
outputs and adds b_o.

All matmuls run in bf16 (full PE rate) with fp32 PSUM accumulation; softmax
runs without max-subtraction (scores are O(10), exp stays in range).

On-device layouts (T = transposed, tokens on the free axis):
  QT/KT: [128 head-dims, 8192 tokens] bf16 in SBUF
  VA:    [128 token-chunk, 64 chunks, 256] bf16; cols 0-127 = V dims,
         cols 128-255 = ones (gives replicated softmax row-sums for free)
  Scores are computed transposed, S.T = [k-tokens, q-tokens], so softmax
  normalization lands on the free axis after the attn@V matmul.

Scheduling (keeps the PE dense so the HAM clock gate stays at 2.4 GHz):
  - Causal masking is applied IN PSUM by a second matmul accumulating
    -384 * U (U = strictly-lower-triangular ones) into the scores, instead
    of multiplying the exp output on GpSimd. This removes the
    PE->Scalar->GpSimd->PE chain on diagonal chunks.
  - The scores/exp pipeline runs two chunks ahead of the attn@V matmuls so
    the ScalarE exp latency is always covered by PE work.
  - Q/K/V projection and output-projection matmuls are chopped into
    single-matmul "filler" units and interleaved between attention chunks,
    so exp-wait bubbles are filled and there is no serialized output-
    projection tail at the end of the kernel.
"""

import sys
import types

sys.path.insert(0, "/opt/trn_rl_repo")

import numpy as np

# Optional: make run_bass_kernel_spmd(trace=True) work on images whose antenv
# lacks axon_hooks. Harmless if unavailable; kernel() defaults to trace=False.
try:  # pragma: no cover
    import antenv
    if "antenv.axon_hooks" not in sys.modules:
        from trn_agent_boot.trn_boot import _ntff_profile_via_ctypes

        _hook = _ntff_profile_via_ctypes("/opt/axon/libaxon_pjrt.so")
        _mod = types.ModuleType("antenv.axon_hooks")
        _mod.get_axon_ntff_profile_hook = lambda: _hook
        _mod.set_axon_ntff_profile_hook = lambda h: None
        sys.modules["antenv.axon_hooks"] = _mod
        antenv.axon_hooks = _mod
except Exception:
    pass

import concourse.bass as bass
import concourse.bacc as bacc
import concourse.tile as tile
import concourse.mybir as mybir
from concourse.bass_utils import run_bass_kernel_spmd

B, S, D, H = 4, 2048, 1024, 16
DK = D // H          # 64
P = 128
SQ = B * S           # 8192 tokens
NT = SQ // 512       # 16 token tiles of 512
KO = D // P          # 8 contraction chunks
NCORES = 8
F16 = mybir.dt.float16
F32 = mybir.dt.float32

TRACE = False        # set by test.py to capture an NTFF profile
LAST_RESULT = None   # BassKernelResults of the most recent run

MM_DT = mybir.dt.bfloat16
MASK_NEG = -384.0    # pre-scale logit bias for masked entries (-48 post-scale)

_NC = None


def _np_mm_dt():
    if MM_DT == mybir.dt.float16:
        return np.float16
    import ml_dtypes
    return ml_dtypes.bfloat16


def _build():
    nc = bacc.Bacc("TRN2", target_bir_lowering=False, debug=False,
                   num_devices=NCORES)

    qT_d = nc.dram_tensor("qT", [NT, P, KO, 512], MM_DT, kind="ExternalInput")
    kT_d = nc.dram_tensor("kT", [NT, P, KO, 512], MM_DT, kind="ExternalInput")
    vT_d = nc.dram_tensor("vT", [NT, P, KO, 512], MM_DT, kind="ExternalInput")
    wq_d = nc.dram_tensor("wq", [P, KO, P], MM_DT, kind="ExternalInput")
    wk_d = nc.dram_tensor("wk", [P, KO, P], MM_DT, kind="ExternalInput")
    wv_d = nc.dram_tensor("wv", [P, KO, P], MM_DT, kind="ExternalInput")
    wo_d = nc.dram_tensor("wo", [P, KO, P], MM_DT, kind="ExternalInput")
    # masks[:, 0, :] = U (strictly-lower-triangular ones, masked positions)
    # masks[:, 1, :] = MASK_NEG * I
    mk_d = nc.dram_tensor("masks", [P, 2, P], MM_DT, kind="ExternalInput")
    out_d = nc.dram_tensor("out", [KO, P, NT, 512], F16, kind="ExternalOutput")

    with tile.TileContext(nc) as tc:
        with (
            tc.tile_pool(name="const", bufs=1) as const,
            tc.tile_pool(name="persist", bufs=1) as persist,
            tc.tile_pool(name="stream", bufs=3) as stream,
            tc.tile_pool(name="epool", bufs=6) as epool,
            tc.tile_pool(name="rpool", bufs=4) as rpool,
            tc.tile_pool(name="ostage", bufs=4) as ostage,
            tc.tile_pool(name="vstage", bufs=2) as vstage,
            tc.tile_pool(name="pp", bufs=2, space="PSUM") as pp,
            tc.tile_pool(name="scp", bufs=4, space="PSUM") as scp,
            tc.tile_pool(name="opp", bufs=2, space="PSUM") as opp,
        ):
            wq_t = const.tile([P, KO, P], MM_DT, tag="wq")
            wk_t = const.tile([P, KO, P], MM_DT, tag="wk")
            wv_t = const.tile([P, KO, P], MM_DT, tag="wv")
            wo_t = const.tile([P, KO, P], MM_DT, tag="wo")
            mk_t = const.tile([P, 2, P], MM_DT, tag="mk")
            nc.sync.dma_start(wq_t[:], wq_d.ap())
            nc.sync.dma_start(wk_t[:], wk_d.ap())
            nc.sync.dma_start(wv_t[:], wv_d.ap())
            nc.sync.dma_start(wo_t[:], wo_d.ap())
            nc.sync.dma_start(mk_t[:], mk_d.ap())

            ident = const.tile([P, P], MM_DT, tag="ident")
            from concourse.masks import make_identity
            make_identity(nc, ident[:])

            QT = persist.tile([P, SQ], MM_DT, tag="QT")
            KT = persist.tile([P, SQ], MM_DT, tag="KT")
            VA = persist.tile([P, SQ // P, 256], MM_DT, tag="VA")
            XT = persist.tile([P, SQ], MM_DT, tag="XT")

            def va_ones():
                # ones columns for the row-sum trick; per-chunk layout is
                # [A dims 0:64 | ones 64:128 | B dims 128:192 | ones 192:256]
                nc.vector.memset(VA[:, :, DK:P], 1.0)
                nc.vector.memset(VA[:, :, P + DK:2 * P], 1.0)

            def proj_units(tt):
                """Q/K/V projections for token tile tt as 28 single-matmul
                filler units. DMA-in starts immediately."""
                cols = bass.ts(tt, 512)
                qin = stream.tile([P, KO, 512], MM_DT, tag="qin")
                nc.sync.dma_start(qin[:], qT_d.ap()[tt])
                kin = stream.tile([P, KO, 512], MM_DT, tag="kin")
                nc.sync.dma_start(kin[:], kT_d.ap()[tt])
                vin = stream.tile([P, KO, 512], MM_DT, tag="vin")
                nc.sync.dma_start(vin[:], vT_d.ap()[tt])

                box = {}

                def mk_mm(w_t, xin, key, ko, final):
                    def u():
                        if ko == 0:
                            box[key] = pp.tile([P, 512], F32, tag="pp",
                                               name="ps_" + key)
                        nc.tensor.matmul(box[key][:], w_t[:, ko, :],
                                         xin[:, ko, :],
                                         start=(ko == 0), stop=(ko == KO - 1))
                        if ko == KO - 1:
                            final(box[key])
                    return u

                def qfin(ps):
                    nc.vector.tensor_copy(QT[:, cols], ps[:])

                def kfin(ps):
                    nc.vector.tensor_copy(KT[:, cols], ps[:])

                def vfin(ps):
                    vts = vstage.tile([P, 512], MM_DT, tag="vts", name="vts")
                    nc.vector.tensor_copy(vts[:], ps[:])
                    box["vts"] = vts

                def mk_tp(sub):
                    def u():
                        tp = pp.tile([P, P], MM_DT, tag="pp", name="tp")
                        nc.tensor.transpose(tp[:],
                                            box["vts"][:, bass.ts(sub, P)],
                                            ident[:])
                        kc = tt * 4 + sub
                        # one strided copy: head halves land at cols 0:64,
                        # 128:192
                        nc.vector.tensor_copy(
                            VA[:, kc].rearrange("p (a x) -> p a x",
                                                a=2)[:, :, 0:DK],
                            tp[:].rearrange("p (a x) -> p a x", a=2))
                    return u

                units = []
                for ko in range(KO):
                    units.append(mk_mm(wq_t, qin, "q", ko, qfin))
                for ko in range(KO):
                    units.append(mk_mm(wk_t, kin, "k", ko, kfin))
                for ko in range(KO):
                    units.append(mk_mm(wv_t, vin, "v", ko, vfin))
                for sub in range(4):
                    units.append(mk_tp(sub))
                return units

            def oproj_units(tt, scalar_cast=False):
                """Output projection for token tile tt: 8 filler units, one
                per 128-wide output-dim chunk (matmul + cast + DMA-out)."""
                def mk(mo):
                    def u():
                        pso = pp.tile([P, 512], F32, tag="pp", name="pso")
                        nc.tensor.matmul(pso[:], wo_t[:, mo, :],
                                         XT[:, bass.ts(tt, 512)],
                                         start=True, stop=True)
                        ost = ostage.tile([P, 512], F16, tag="ost",
                                          name="ost")
                        if scalar_cast and (mo % 2 == 1):
                            nc.scalar.activation(
                                ost[:], pso[:],
                                mybir.ActivationFunctionType.Copy)
                        else:
                            nc.vector.tensor_copy(ost[:], pso[:])
                        nc.sync.dma_start(out_d.ap()[mo, :, tt, :], ost[:])
                    return u
                return [mk(mo) for mo in range(KO)]

            def attention(b, qt, fillers):
                """One 512-query tile of causal attention, both heads.

                scores+exp run two k-chunks ahead of the attn@V matmuls;
                filler units are popped between chunks to keep the PE dense
                while ScalarE works through the exps.
                """
                qcols = bass.ds(b * S + qt * 512, 512)
                nkc = 4 * qt + 4
                ops = [opp.tile([P, 512], F32, tag="op", name=f"op{h}")
                       for h in range(2)]
                e_box = {}

                def scores_exp(kc):
                    kcols = bass.ds(b * S + kc * P, P)
                    j = kc - 4 * qt
                    diag = j >= 0
                    co = max(j, 0) * P  # valid q-columns start here (causal)
                    w = 512 - co
                    pair = []
                    for h in range(2):
                        rb = h * DK
                        ssc = scp.tile([P, 512], F32, tag="sc", name="ssc")
                        nc.tensor.matmul(
                            ssc[:, co:],
                            KT[rb:rb + DK, kcols],
                            QT[rb:rb + DK, bass.ds(b * S + qt * 512 + co, w)],
                            start=True, stop=not diag)
                        if diag:
                            # accumulate -384 * U into the diagonal block:
                            # masked (q < k) logits drop to ~-48 post-scale
                            nc.tensor.matmul(
                                ssc[:, co:co + P],
                                mk_t[:, 1, :], mk_t[:, 0, :],
                                start=False, stop=True)
                        e_t = epool.tile([P, 512], MM_DT, tag="e", name="e_t")
                        nc.scalar.activation(e_t[:, co:], ssc[:, co:],
                                             mybir.ActivationFunctionType.Exp,
                                             scale=0.125)
                        pair.append((e_t, co))
                    e_box[kc] = pair

                def attn_mm(kc):
                    gkc = b * (S // P) + kc
                    pair = e_box.pop(kc)
                    for h in range(2):
                        e_t, co = pair[h]
                        nc.tensor.matmul(ops[h][:, co:],
                                         VA[:, gkc, bass.ts(h, P)],
                                         e_t[:, co:],
                                         start=(kc == 0), stop=(kc == nkc - 1))

                def pop_fillers(kc):
                    left = nkc - kc
                    n = (len(fillers) + left - 1) // left
                    for _ in range(n):
                        if fillers:
                            fillers.pop(0)()

                scores_exp(0)
                if nkc > 1:
                    scores_exp(1)
                for kc in range(nkc):
                    if kc + 2 < nkc:
                        scores_exp(kc + 2)
                    pop_fillers(kc)
                    attn_mm(kc)
                while fillers:
                    fillers.pop(0)()

                for h in range(2):
                    r_t = rpool.tile([DK, 512], F32, tag="r", name="r_t")
                    s_t = rpool.tile([DK, 512], F32, tag="s", name="s_t")
                    nc.vector.tensor_copy(s_t[:], ops[h][DK:P, :])
                    nc.vector.reciprocal_approx_fast(r_t[:], s_t[:])
                    nc.vector.tensor_mul(XT[h * DK:(h + 1) * DK, qcols],
                                         ops[h][0:DK, :], r_t[:])

            # Batch-0 projections up front (they cover the DMA-in latency and
            # warm the HAM clock gate); everything else is interleaved into
            # the attention stream as single-matmul fillers.
            for u in proj_units(0):
                u()
            va_ones()
            for tt in range(1, 4):
                for u in proj_units(tt):
                    u()
            for g in range(NT):
                b, qt = divmod(g, 4)
                fillers = []
                if g + 4 < NT:
                    fillers += proj_units(g + 4)
                if g >= 1:
                    fillers += oproj_units(g - 1)
                attention(b, qt, fillers)
            for u in oproj_units(NT - 1, scalar_cast=True):
                u()

    nc.compile()
    return nc


def _get_nc():
    global _NC
    if _NC is None:
        _NC = _build()
    return _NC


def _to_tiled_T(x2):
    """[SQ, D] fp32 -> [NT, 128, KO, 512] bf16 with x[g, d] at
    [g//512, d%128, d//128, g%512]."""
    xh = x2.astype(_np_mm_dt())
    return np.ascontiguousarray(
        xh.reshape(NT, 512, KO, P).transpose(0, 3, 2, 1))


def _weight_T(w_slice):
    """[128 out, 1024 in] -> [128 p, KO, 128 m] bf16 with W[m, d] at
    [d%128, d//128, m]."""
    return np.ascontiguousarray(
        w_slice.T.reshape(KO, P, P).transpose(1, 0, 2)).astype(_np_mm_dt())


def kernel(q, k, v, mask, W_q, W_k, W_v, W_o, b_o):
    global LAST_RESULT
    nc = _get_nc()

    qT = _to_tiled_T(np.asarray(q, np.float32).reshape(SQ, D))
    kT = _to_tiled_T(np.asarray(k, np.float32).reshape(SQ, D))
    vT = _to_tiled_T(np.asarray(v, np.float32).reshape(SQ, D))

    p_idx = np.arange(P)[:, None]
    f_idx = np.arange(P)[None, :]
    U = (f_idx < p_idx).astype(np.float32)           # masked: q < k
    Dneg = MASK_NEG * np.eye(P, dtype=np.float32)
    masks = np.stack([U, Dneg], axis=1).astype(_np_mm_dt())  # [P, 2, P]

    W_q = np.asarray(W_q, np.float32)
    W_k = np.asarray(W_k, np.float32)
    W_v = np.asarray(W_v, np.float32)
    W_o = np.asarray(W_o, np.float32)

    in_maps = []
    for c in range(NCORES):
        cs = slice(c * P, (c + 1) * P)
        in_maps.append({
            "qT": qT, "kT": kT, "vT": vT, "masks": masks,
            "wq": _weight_T(W_q[cs, :]),
            "wk": _weight_T(W_k[cs, :]),
            "wv": _weight_T(W_v[cs, :]),
            # [k, mo, m] = W_o[mo*128+m, c*128+k]
            "wo": np.ascontiguousarray(
                W_o[:, cs].reshape(KO, P, P).transpose(2, 0, 1)
            ).astype(_np_mm_dt()),
        })

    res = run_bass_kernel_spmd(nc, in_maps, core_ids=list(range(NCORES)),
                               trace=TRACE)
    LAST_RESULT = res

    acc = np.zeros((SQ, D), np.float32)
    for c in range(NCORES):
        partial_T = res.results[c]["out"].reshape(D, SQ)
        acc += partial_T.T.astype(np.float32)
    acc += np.asarray(b_o, np.float32)
    return acc.reshape(B, S, D)


# revision 17
# speedup vs baseline: 1.0039x; 1.0039x over previous
"""Multi-head attention (B=4, S=2048, D=1024, H=16) on 8 Trainium2 NeuronCores.

Sharding: tensor-parallel over heads. Core c owns heads 2c, 2c+1 (a 128-wide
slice of the model dim). Each core computes Q/K/V projections for its head
slice over all tokens, causal attention for its 2 heads, and a partial output
projection (contraction over its 128 x-dims). The host sums the 8 partial
outputs and adds b_o.

All matmuls run in bf16 (full PE rate) with fp32 PSUM accumulation; softmax
runs without max-subtraction (scores are O(10), exp stays in range).

On-device layouts (T = transposed, tokens on the free axis):
  QT/KT: [128 head-dims, 8192 tokens] bf16 in SBUF
  VA:    [128 token-chunk, 64 chunks, 256] bf16; cols 0-127 = V dims,
         cols 128-255 = ones (gives replicated softmax row-sums for free)
  Scores are computed transposed, S.T = [k-tokens, q-tokens], so softmax
  normalization lands on the free axis after the attn@V matmul.

Scheduling (keeps the PE dense so the HAM clock gate stays at 2.4 GHz):
  - The scores/exp pipeline runs two chunks ahead of the attn@V matmuls so
    the ScalarE exp (and the GpSimd mask multiply on diagonal chunks) is
    always covered by PE work.
  - Q/K/V projection and output-projection matmuls are chopped into
    single-matmul "filler" units and interleaved between attention chunks,
    so exp-wait bubbles are filled and there is no serialized output-
    projection tail at the end of the kernel.

"""

import sys
import types

sys.path.insert(0, "/opt/trn_rl_repo")

import numpy as np

# Optional: make run_bass_kernel_spmd(trace=True) work on images whose antenv
# lacks axon_hooks. Harmless if unavailable; kernel() defaults to trace=False.
try:  # pragma: no cover
    import antenv
    if "antenv.axon_hooks" not in sys.modules:
        from trn_agent_boot.trn_boot import _ntff_profile_via_ctypes

        _hook = _ntff_profile_via_ctypes("/opt/axon/libaxon_pjrt.so")
        _mod = types.ModuleType("antenv.axon_hooks")
        _mod.get_axon_ntff_profile_hook = lambda: _hook
        _mod.set_axon_ntff_profile_hook = lambda h: None
        sys.modules["antenv.axon_hooks"] = _mod
        antenv.axon_hooks = _mod
except Exception:
    pass

import concourse.bass as bass
import concourse.bacc as bacc
import concourse.tile as tile
import concourse.mybir as mybir
from concourse.bass_utils import run_bass_kernel_spmd

B, S, D, H = 4, 2048, 1024, 16
DK = D // H          # 64
P = 128
SQ = B * S           # 8192 tokens
NT = SQ // 512       # 16 token tiles of 512
KO = D // P          # 8 contraction chunks
NCORES = 8
F16 = mybir.dt.float16
F32 = mybir.dt.float32

TRACE = False        # set by test.py to capture an NTFF profile
LAST_RESULT = None   # BassKernelResults of the most recent run

MM_DT = mybir.dt.bfloat16

_NC = None


def _np_mm_dt():
    if MM_DT == mybir.dt.float16:
        return np.float16
    import ml_dtypes
    return ml_dtypes.bfloat16


def _build():
    nc = bacc.Bacc("TRN2", target_bir_lowering=False, debug=False,
                   num_devices=NCORES)

    qT_d = nc.dram_tensor("qT", [NT, P, KO, 512], MM_DT, kind="ExternalInput")
    kT_d = nc.dram_tensor("kT", [NT, P, KO, 512], MM_DT, kind="ExternalInput")
    vT_d = nc.dram_tensor("vT", [NT, P, KO, 512], MM_DT, kind="ExternalInput")
    wq_d = nc.dram_tensor("wq", [P, KO, P], MM_DT, kind="ExternalInput")
    wk_d = nc.dram_tensor("wk", [P, KO, P], MM_DT, kind="ExternalInput")
    wv_d = nc.dram_tensor("wv", [P, KO, P], MM_DT, kind="ExternalInput")
    wo_d = nc.dram_tensor("wo", [P, KO, P], MM_DT, kind="ExternalInput")
    mk_d = nc.dram_tensor("masks", [P, P], MM_DT, kind="ExternalInput")
    out_d = nc.dram_tensor("out", [KO, P, NT, 512], F16, kind="ExternalOutput")

    with tile.TileContext(nc) as tc:
        with (
            tc.tile_pool(name="const", bufs=1) as const,
            tc.tile_pool(name="persist", bufs=1) as persist,
            tc.tile_pool(name="stream", bufs=3) as stream,
            tc.tile_pool(name="epool", bufs=6) as epool,
            tc.tile_pool(name="rpool", bufs=4) as rpool,
            tc.tile_pool(name="ostage", bufs=4) as ostage,
            tc.tile_pool(name="vstage", bufs=2) as vstage,
            tc.tile_pool(name="pp", bufs=2, space="PSUM") as pp,
            tc.tile_pool(name="scp", bufs=4, space="PSUM") as scp,
            tc.tile_pool(name="opp", bufs=2, space="PSUM") as opp,
        ):
            wq_t = const.tile([P, KO, P], MM_DT, tag="wq")
            wk_t = const.tile([P, KO, P], MM_DT, tag="wk")
            wv_t = const.tile([P, KO, P], MM_DT, tag="wv")
            wo_t = const.tile([P, KO, P], MM_DT, tag="wo")
            mk_t = const.tile([P, P], MM_DT, tag="mk")
            nc.sync.dma_start(wq_t[:], wq_d.ap())
            nc.sync.dma_start(wk_t[:], wk_d.ap())
            nc.sync.dma_start(wv_t[:], wv_d.ap())
            nc.sync.dma_start(wo_t[:], wo_d.ap())
            nc.sync.dma_start(mk_t[:], mk_d.ap())

            ident = const.tile([P, P], MM_DT, tag="ident")
            from concourse.masks import make_identity
            make_identity(nc, ident[:])

            QT = persist.tile([P, SQ], MM_DT, tag="QT")
            KT = persist.tile([P, SQ], MM_DT, tag="KT")
            VA = persist.tile([P, SQ // P, 256], MM_DT, tag="VA")
            XT = persist.tile([P, SQ], MM_DT, tag="XT")

            def va_ones():
                # ones columns for the row-sum trick; per-chunk layout is
                # [A dims 0:64 | ones 64:128 | B dims 128:192 | ones 192:256]
                nc.vector.memset(VA[:, :, DK:P], 1.0)
                nc.vector.memset(VA[:, :, P + DK:2 * P], 1.0)

            def proj_units(tt):
                """Q/K/V projections for token tile tt as 28 single-matmul
                filler units. DMA-in starts immediately."""
                cols = bass.ts(tt, 512)
                qin = stream.tile([P, KO, 512], MM_DT, tag="qin")
                nc.sync.dma_start(qin[:], qT_d.ap()[tt])
                kin = stream.tile([P, KO, 512], MM_DT, tag="kin")
                nc.sync.dma_start(kin[:], kT_d.ap()[tt])
                vin = stream.tile([P, KO, 512], MM_DT, tag="vin")
                nc.sync.dma_start(vin[:], vT_d.ap()[tt])

                box = {}

                def mk_mm(w_t, xin, key, ko, final):
                    def u():
                        if ko == 0:
                            box[key] = pp.tile([P, 512], F32, tag="pp",
                                               name="ps_" + key)
                        nc.tensor.matmul(box[key][:], w_t[:, ko, :],
                                         xin[:, ko, :],
                                         start=(ko == 0), stop=(ko == KO - 1))
                        if ko == KO - 1:
                            final(box[key])
                    return u

                def qfin(ps):
                    nc.vector.tensor_copy(QT[:, cols], ps[:])

                def kfin(ps):
                    nc.vector.tensor_copy(KT[:, cols], ps[:])

                def vfin(ps):
                    vts = vstage.tile([P, 512], MM_DT, tag="vts", name="vts")
                    nc.vector.tensor_copy(vts[:], ps[:])
                    box["vts"] = vts

                def mk_tp(sub):
                    def u():
                        tp = pp.tile([P, P], MM_DT, tag="pp", name="tp")
                        nc.tensor.transpose(tp[:],
                                            box["vts"][:, bass.ts(sub, P)],
                                            ident[:])
                        kc = tt * 4 + sub
                        # one strided copy: head halves land at cols 0:64,
                        # 128:192
                        nc.vector.tensor_copy(
                            VA[:, kc].rearrange("p (a x) -> p a x",
                                                a=2)[:, :, 0:DK],
                            tp[:].rearrange("p (a x) -> p a x", a=2))
                    return u

                units = []
                for ko in range(KO):
                    units.append(mk_mm(wq_t, qin, "q", ko, qfin))
                for ko in range(KO):
                    units.append(mk_mm(wk_t, kin, "k", ko, kfin))
                for ko in range(KO):
                    units.append(mk_mm(wv_t, vin, "v", ko, vfin))
                for sub in range(4):
                    units.append(mk_tp(sub))
                return units

            def oproj_units(tt, scalar_cast=False):
                """Output projection for token tile tt: 8 filler units, one
                per 128-wide output-dim chunk (matmul + cast + DMA-out)."""
                def mk(mo):
                    def u():
                        pso = pp.tile([P, 512], F32, tag="pp", name="pso")
                        nc.tensor.matmul(pso[:], wo_t[:, mo, :],
                                         XT[:, bass.ts(tt, 512)],
                                         start=True, stop=True)
                        ost = ostage.tile([P, 512], F16, tag="ost",
                                          name="ost")
                        if scalar_cast and (mo % 2 == 1):
                            nc.scalar.activation(
                                ost[:], pso[:],
                                mybir.ActivationFunctionType.Copy)
                        else:
                            nc.vector.tensor_copy(ost[:], pso[:])
                        nc.sync.dma_start(out_d.ap()[mo, :, tt, :], ost[:])
                    return u
                return [mk(mo) for mo in range(KO)]

            def attention(b, qt, fillers):
                """One 512-query tile of causal attention, both heads.

                scores+exp run two k-chunks ahead of the attn@V matmuls;
                filler units are popped between chunks to keep the PE dense
                while ScalarE works through the exps.
                """
                qcols = bass.ds(b * S + qt * 512, 512)
                nkc = 4 * qt + 4
                ops = [opp.tile([P, 512], F32, tag="op", name=f"op{h}")
                       for h in range(2)]
                e_box = {}

                def scores_exp(kc):
                    kcols = bass.ds(b * S + kc * P, P)
                    j = kc - 4 * qt
                    co = max(j, 0) * P  # valid q-columns start here (causal)
                    w = 512 - co
                    pair = []
                    for h in range(2):
                        rb = h * DK
                        ssc = scp.tile([P, 512], F32, tag="sc", name="ssc")
                        nc.tensor.matmul(
                            ssc[:, co:],
                            KT[rb:rb + DK, kcols],
                            QT[rb:rb + DK, bass.ds(b * S + qt * 512 + co, w)],
                            start=True, stop=True)
                        e_t = epool.tile([P, 512], MM_DT, tag="e", name="e_t")
                        nc.scalar.activation(e_t[:, co:], ssc[:, co:],
                                             mybir.ActivationFunctionType.Exp,
                                             scale=0.125)
                        if j >= 0:
                            nc.gpsimd.tensor_mul(e_t[:, co:co + P],
                                                 e_t[:, co:co + P], mk_t[:])
                        pair.append((e_t, co))
                    e_box[kc] = pair

                def attn_mm(kc):
                    gkc = b * (S // P) + kc
                    pair = e_box.pop(kc)
                    for h in range(2):
                        e_t, co = pair[h]
                        nc.tensor.matmul(ops[h][:, co:],
                                         VA[:, gkc, bass.ts(h, P)],
                                         e_t[:, co:],
                                         start=(kc == 0), stop=(kc == nkc - 1))

                def pop_fillers(kc):
                    left = nkc - kc
                    n = (len(fillers) + left - 1) // left
                    for _ in range(n):
                        if fillers:
                            fillers.pop(0)()

                scores_exp(0)
                if nkc > 1:
                    scores_exp(1)
                for kc in range(nkc):
                    if kc + 2 < nkc:
                        scores_exp(kc + 2)
                    pop_fillers(kc)
                    attn_mm(kc)
                while fillers:
                    fillers.pop(0)()

                for h in range(2):
                    r_t = rpool.tile([DK, 512], F32, tag="r", name="r_t")
                    s_t = rpool.tile([DK, 512], F32, tag="s", name="s_t")
                    nc.vector.tensor_copy(s_t[:], ops[h][DK:P, :])
                    nc.vector.reciprocal_approx_fast(r_t[:], s_t[:])
                    nc.vector.tensor_mul(XT[h * DK:(h + 1) * DK, qcols],
                                         ops[h][0:DK, :], r_t[:])

            # Batch-0 projections up front (they cover the DMA-in latency and
            # warm the HAM clock gate); everything else is interleaved into
            # the attention stream as single-matmul fillers. The assignment
            # pushes projection sets as late as their deadlines allow and
            # reserves output-projection sets for the late tiles, so the
            # exp-paced qt=3 tiles (especially batch 3, which has no
            # projection work left) still keep the PE dense and the HAM
            # clock gate at 8/8.
            PROJ_AT = {0: [4, 5], 1: [6, 7, 8], 2: [9], 3: [10],
                       6: [11], 7: [12], 10: [13], 11: [14], 14: [15]}
            OPROJ_AT = {3: [0], 4: [1], 5: [2, 3], 7: [4], 8: [5],
                        9: [6, 7], 11: [8], 12: [9], 13: [10, 11],
                        15: [12, 13, 14]}
            for u in proj_units(0):
                u()
            va_ones()
            for tt in range(1, 4):
                for u in proj_units(tt):
                    u()
            for g in range(NT):
                b, qt = divmod(g, 4)
                fillers = []
                for t in PROJ_AT.get(g, []):
                    fillers += proj_units(t)
                for t in OPROJ_AT.get(g, []):
                    fillers += oproj_units(t)
                attention(b, qt, fillers)
            for u in oproj_units(NT - 1, scalar_cast=True):
                u()

    nc.compile()
    return nc


def _get_nc():
    global _NC
    if _NC is None:
        _NC = _build()
    return _NC


def _to_tiled_T(x2):
    """[SQ, D] fp32 -> [NT, 128, KO, 512] bf16 with x[g, d] at
    [g//512, d%128, d//128, g%512]."""
    xh = x2.astype(_np_mm_dt())
    return np.ascontiguousarray(
        xh.reshape(NT, 512, KO, P).transpose(0, 3, 2, 1))


def _weight_T(w_slice):
    """[128 out, 1024 in] -> [128 p, KO, 128 m] bf16 with W[m, d] at
    [d%128, d//128, m]."""
    return np.ascontiguousarray(
        w_slice.T.reshape(KO, P, P).transpose(1, 0, 2)).astype(_np_mm_dt())


def kernel(q, k, v, mask, W_q, W_k, W_v, W_o, b_o):
    global LAST_RESULT
    nc = _get_nc()

    qT = _to_tiled_T(np.asarray(q, np.float32).reshape(SQ, D))
    kT = _to_tiled_T(np.asarray(k, np.float32).reshape(SQ, D))
    vT = _to_tiled_T(np.asarray(v, np.float32).reshape(SQ, D))

    p_idx = np.arange(P)[:, None]
    f_idx = np.arange(P)[None, :]
    masks = (f_idx >= p_idx).astype(_np_mm_dt())

    W_q = np.asarray(W_q, np.float32)
    W_k = np.asarray(W_k, np.float32)
    W_v = np.asarray(W_v, np.float32)
    W_o = np.asarray(W_o, np.float32)

    in_maps = []
    for c in range(NCORES):
        cs = slice(c * P, (c + 1) * P)
        in_maps.append({
            "qT": qT, "kT": kT, "vT": vT, "masks": masks,
            "wq": _weight_T(W_q[cs, :]),
            "wk": _weight_T(W_k[cs, :]),
            "wv": _weight_T(W_v[cs, :]),
            # [k, mo, m] = W_o[mo*128+m, c*128+k]
            "wo": np.ascontiguousarray(
                W_o[:, cs].reshape(KO, P, P).transpose(2, 0, 1)
            ).astype(_np_mm_dt()),
        })

    res = run_bass_kernel_spmd(nc, in_maps, core_ids=list(range(NCORES)),
                               trace=TRACE)
    LAST_RESULT = res

    acc = np.zeros((SQ, D), np.float32)
    for c in range(NCORES):
        partial_T = res.results[c]["out"].reshape(D, SQ)
        acc += partial_T.T.astype(np.float32)
    acc += np.asarray(b_o, np.float32)
    return acc.reshape(B, S, D)


# revision 23
# speedup vs baseline: 1.0646x; 1.0604x over previous
"""Multi-head attention (B=4, S=2048, D=1024, H=16) on 8 Trainium2 NeuronCores.

Sharding: tensor-parallel over heads. Core c owns heads 2c, 2c+1 (a 128-wide
slice of the model dim). Each core computes Q/K/V projections for its head
slice over all tokens, causal attention for its 2 heads, and a partial output
projection (contraction over its 128 x-dims). The host sums the 8 partial
outputs and adds b_o.

All matmuls run in bf16 (full PE rate) with fp32 PSUM accumulation; softmax
runs without max-subtraction (scores are O(10), exp stays in range).

On-device layouts (T = transposed, tokens on the free axis):
  QT/KT: [128 head-dims, 8192 tokens] bf16 in SBUF
  VA:    [128 token-chunk, 64 chunks, 256] bf16; cols 0-127 = V dims,
         cols 128-255 = ones (gives replicated softmax row-sums for free)
  Scores are computed transposed, S.T = [k-tokens, q-tokens], so softmax
  normalization lands on the free axis after the attn@V matmul.

Scheduling (keeps the PE dense so the HAM clock gate stays at 2.4 GHz):
  - The scores/exp pipeline runs two chunks ahead of the attn@V matmuls so
    the ScalarE exp (and the GpSimd mask multiply on diagonal chunks) is
    always covered by PE work.
  - Q/K/V projection and output-projection matmuls are chopped into
    single-matmul "filler" units and interleaved between attention chunks,
    so exp-wait bubbles are filled and there is no serialized output-
    projection tail at the end of the kernel.

"""

import sys
import types

sys.path.insert(0, "/opt/trn_rl_repo")

import numpy as np

# Optional: make run_bass_kernel_spmd(trace=True) work on images whose antenv
# lacks axon_hooks. Harmless if unavailable; kernel() defaults to trace=False.
try:  # pragma: no cover
    import antenv
    if "antenv.axon_hooks" not in sys.modules:
        from trn_agent_boot.trn_boot import _ntff_profile_via_ctypes

        _hook = _ntff_profile_via_ctypes("/opt/axon/libaxon_pjrt.so")
        _mod = types.ModuleType("antenv.axon_hooks")
        _mod.get_axon_ntff_profile_hook = lambda: _hook
        _mod.set_axon_ntff_profile_hook = lambda h: None
        sys.modules["antenv.axon_hooks"] = _mod
        antenv.axon_hooks = _mod
except Exception:
    pass

import concourse.bass as bass
import concourse.bacc as bacc
import concourse.tile as tile
import concourse.mybir as mybir
from concourse.bass_utils import run_bass_kernel_spmd

B, S, D, H = 4, 2048, 1024, 16
DK = D // H          # 64
P = 128
SQ = B * S           # 8192 tokens
NT = SQ // 512       # 16 token tiles of 512
KO = D // P          # 8 contraction chunks
NCORES = 8
F16 = mybir.dt.float16
F32 = mybir.dt.float32

TRACE = False        # set by test.py to capture an NTFF profile
LAST_RESULT = None   # BassKernelResults of the most recent run

MM_DT = mybir.dt.bfloat16

_NC = None


def _np_mm_dt():
    if MM_DT == mybir.dt.float16:
        return np.float16
    import ml_dtypes
    return ml_dtypes.bfloat16


def _build():
    nc = bacc.Bacc("TRN2", target_bir_lowering=False, debug=False,
                   num_devices=NCORES)

    qT_d = nc.dram_tensor("qT", [NT, P, KO, 512], MM_DT, kind="ExternalInput")
    kT_d = nc.dram_tensor("kT", [NT, P, KO, 512], MM_DT, kind="ExternalInput")
    vT_d = nc.dram_tensor("vT", [NT, P, KO, 512], MM_DT, kind="ExternalInput")
    wq_d = nc.dram_tensor("wq", [P, KO, P], MM_DT, kind="ExternalInput")
    wk_d = nc.dram_tensor("wk", [P, KO, P], MM_DT, kind="ExternalInput")
    wv_d = nc.dram_tensor("wv", [P, KO, P], MM_DT, kind="ExternalInput")
    wo_d = nc.dram_tensor("wo", [P, KO, P], MM_DT, kind="ExternalInput")
    # masks[:, 0, :] = U (strictly-lower-triangular ones, masked positions)
    # masks[:, 1, :] = -384 * I  (bias matmul: masked logits -> -48 post-scale)
    mk_d = nc.dram_tensor("masks", [P, 2, P], MM_DT, kind="ExternalInput")
    out_d = nc.dram_tensor("out", [KO, P, NT, 512], F16, kind="ExternalOutput")

    with tile.TileContext(nc) as tc:
        with (
            tc.tile_pool(name="const", bufs=1) as const,
            tc.tile_pool(name="persist", bufs=1) as persist,
            tc.tile_pool(name="stream", bufs=3) as stream,
            tc.tile_pool(name="epool", bufs=8) as epool,
            tc.tile_pool(name="rpool", bufs=4) as rpool,
            tc.tile_pool(name="ostage", bufs=4) as ostage,
            tc.tile_pool(name="vstage", bufs=2) as vstage,
            tc.tile_pool(name="pp", bufs=2, space="PSUM") as pp,
            tc.tile_pool(name="scp", bufs=4, space="PSUM") as scp,
            tc.tile_pool(name="opp", bufs=2, space="PSUM") as opp,
        ):
            wq_t = const.tile([P, KO, P], MM_DT, tag="wq")
            wk_t = const.tile([P, KO, P], MM_DT, tag="wk")
            wv_t = const.tile([P, KO, P], MM_DT, tag="wv")
            wo_t = const.tile([P, KO, P], MM_DT, tag="wo")
            mk_t = const.tile([P, 2, P], MM_DT, tag="mk")
            nc.sync.dma_start(wq_t[:], wq_d.ap())
            nc.sync.dma_start(wk_t[:], wk_d.ap())
            nc.sync.dma_start(wv_t[:], wv_d.ap())
            nc.sync.dma_start(wo_t[:], wo_d.ap())
            nc.sync.dma_start(mk_t[:], mk_d.ap())

            ident = const.tile([P, P], MM_DT, tag="ident")
            from concourse.masks import make_identity
            make_identity(nc, ident[:])

            QT = persist.tile([P, SQ], MM_DT, tag="QT")
            KT = persist.tile([P, SQ], MM_DT, tag="KT")
            VA = persist.tile([P, SQ // P, 256], MM_DT, tag="VA")
            XT = persist.tile([P, SQ], MM_DT, tag="XT")

            def va_ones():
                # ones columns for the row-sum trick; per-chunk layout is
                # [A dims 0:64 | ones 64:128 | B dims 128:192 | ones 192:256]
                nc.vector.memset(VA[:, :, DK:P], 1.0)
                nc.vector.memset(VA[:, :, P + DK:2 * P], 1.0)

            def proj_units(tt):
                """Q/K/V projections for token tile tt as 28 single-matmul
                filler units. DMA-in starts immediately."""
                cols = bass.ts(tt, 512)
                qin = stream.tile([P, KO, 512], MM_DT, tag="qin")
                nc.sync.dma_start(qin[:], qT_d.ap()[tt])
                kin = stream.tile([P, KO, 512], MM_DT, tag="kin")
                nc.sync.dma_start(kin[:], kT_d.ap()[tt])
                vin = stream.tile([P, KO, 512], MM_DT, tag="vin")
                nc.sync.dma_start(vin[:], vT_d.ap()[tt])

                box = {}

                def mk_mm(w_t, xin, key, ko, final):
                    def u():
                        if ko == 0:
                            box[key] = pp.tile([P, 512], F32, tag="pp",
                                               name="ps_" + key)
                        nc.tensor.matmul(box[key][:], w_t[:, ko, :],
                                         xin[:, ko, :],
                                         start=(ko == 0), stop=(ko == KO - 1))
                        if ko == KO - 1:
                            final(box[key])
                    return u

                def qfin(ps):
                    nc.vector.tensor_copy(QT[:, cols], ps[:])

                def kfin(ps):
                    nc.vector.tensor_copy(KT[:, cols], ps[:])

                def vfin(ps):
                    vts = vstage.tile([P, 512], MM_DT, tag="vts", name="vts")
                    nc.vector.tensor_copy(vts[:], ps[:])
                    box["vts"] = vts

                def mk_tp(sub):
                    def u():
                        tp = pp.tile([P, P], MM_DT, tag="pp", name="tp")
                        nc.tensor.transpose(tp[:],
                                            box["vts"][:, bass.ts(sub, P)],
                                            ident[:])
                        kc = tt * 4 + sub
                        # one strided copy: head halves land at cols 0:64,
                        # 128:192
                        nc.vector.tensor_copy(
                            VA[:, kc].rearrange("p (a x) -> p a x",
                                                a=2)[:, :, 0:DK],
                            tp[:].rearrange("p (a x) -> p a x", a=2))
                    return u

                units = []
                for ko in range(KO):
                    units.append(mk_mm(wq_t, qin, "q", ko, qfin))
                for ko in range(KO):
                    units.append(mk_mm(wk_t, kin, "k", ko, kfin))
                for ko in range(KO):
                    units.append(mk_mm(wv_t, vin, "v", ko, vfin))
                for sub in range(4):
                    units.append(mk_tp(sub))
                return units

            def oproj_units(tt, scalar_cast=False):
                """Output projection for token tile tt: 8 filler units, one
                per 128-wide output-dim chunk (matmul + cast + DMA-out)."""
                def mk(mo):
                    def u():
                        pso = pp.tile([P, 512], F32, tag="pp", name="pso")
                        nc.tensor.matmul(pso[:], wo_t[:, mo, :],
                                         XT[:, bass.ts(tt, 512)],
                                         start=True, stop=True)
                        ost = ostage.tile([P, 512], F16, tag="ost",
                                          name="ost")
                        if scalar_cast and (mo % 2 == 1):
                            nc.scalar.activation(
                                ost[:], pso[:],
                                mybir.ActivationFunctionType.Copy)
                        else:
                            nc.vector.tensor_copy(ost[:], pso[:])
                        nc.sync.dma_start(out_d.ap()[mo, :, tt, :], ost[:])
                    return u
                return [mk(mo) for mo in range(KO)]

            def attention(b, qt, fillers):
                """One 512-query tile of causal attention, both heads.

                scores+exp run two k-chunks ahead of the attn@V matmuls, in
                groups of two chunks: the 4 row-tiled score matmuls of a
                group are emitted back-to-back (entering/leaving row-tiled
                mode costs ~100-190ns, so batch the transitions), then the
                full-array work (mask-bias matmuls, fillers, attn@V).
                Filler units keep the PE dense while ScalarE works through
                the exps.
                """
                qcols = bass.ds(b * S + qt * 512, 512)
                nkc = 4 * qt + 4
                ops = [opp.tile([P, 512], F32, tag="op", name=f"op{h}")
                       for h in range(2)]
                s_box = {}
                e_box = {}

                def sc_mm(kc):
                    kcols = bass.ds(b * S + kc * P, P)
                    j = kc - 4 * qt
                    co = max(j, 0) * P  # valid q-columns start here (causal)
                    w = 512 - co
                    pair = []
                    for h in range(2):
                        rb = h * DK
                        ssc = scp.tile([P, 512], F32, tag="sc", name="ssc")
                        nc.tensor.matmul(
                            ssc[:, co:],
                            KT[rb:rb + DK, kcols],
                            QT[rb:rb + DK, bass.ds(b * S + qt * 512 + co, w)],
                            start=True, stop=(j < 0))
                        pair.append(ssc)
                    s_box[kc] = (pair, co, j >= 0)

                def mask_exp(kc):
                    pair, co, diag = s_box.pop(kc)
                    epair = []
                    for h in range(2):
                        ssc = pair[h]
                        if diag:
                            # accumulate -384 * U into the diagonal block:
                            # masked (q < k) logits drop to ~-48 post-scale
                            nc.tensor.matmul(
                                ssc[:, co:co + P],
                                mk_t[:, 1, :], mk_t[:, 0, :],
                                start=False, stop=True)
                        e_t = epool.tile([P, 512], MM_DT, tag="e", name="e_t")
                        nc.scalar.activation(e_t[:, co:], ssc[:, co:],
                                             mybir.ActivationFunctionType.Exp,
                                             scale=0.125)
                        epair.append((e_t, co))
                    e_box[kc] = epair

                def attn_mm(kc):
                    gkc = b * (S // P) + kc
                    pair = e_box.pop(kc)
                    for h in range(2):
                        e_t, co = pair[h]
                        nc.tensor.matmul(ops[h][:, co:],
                                         VA[:, gkc, bass.ts(h, P)],
                                         e_t[:, co:],
                                         start=(kc == 0), stop=(kc == nkc - 1))

                def pop_fillers(kc):
                    left = nkc - kc
                    n = (len(fillers) * 2 + left - 1) // left
                    for _ in range(n):
                        if fillers:
                            fillers.pop(0)()

                sc_mm(0)
                sc_mm(1)
                mask_exp(0)
                mask_exp(1)
                for kc in range(0, nkc, 2):
                    if kc + 2 < nkc:
                        sc_mm(kc + 2)
                        sc_mm(kc + 3)
                        mask_exp(kc + 2)
                        mask_exp(kc + 3)
                    pop_fillers(kc)
                    attn_mm(kc)
                    attn_mm(kc + 1)
                while fillers:
                    fillers.pop(0)()

                for h in range(2):
                    r_t = rpool.tile([DK, 512], F32, tag="r", name="r_t")
                    s_t = rpool.tile([DK, 512], F32, tag="s", name="s_t")
                    nc.vector.tensor_copy(s_t[:], ops[h][DK:P, :])
                    nc.vector.reciprocal_approx_fast(r_t[:], s_t[:])
                    nc.vector.tensor_mul(XT[h * DK:(h + 1) * DK, qcols],
                                         ops[h][0:DK, :], r_t[:])

            # Batch-0 projections up front (they cover the DMA-in latency and
            # warm the HAM clock gate); everything else is interleaved into
            # the attention stream as single-matmul fillers. The assignment
            # pushes projection sets as late as their deadlines allow and
            # reserves output-projection sets for the late tiles, so the
            # exp-paced qt=3 tiles (especially batch 3, which has no
            # projection work left) still keep the PE dense and the HAM
            # clock gate at 8/8.
            PROJ_AT = {0: [4, 5], 1: [6, 7, 8], 2: [9], 3: [10],
                       6: [11], 7: [12], 10: [13], 11: [14], 14: [15]}
            OPROJ_AT = {3: [0], 4: [1], 5: [2, 3], 7: [4], 8: [5],
                        9: [6, 7], 11: [8], 12: [9], 13: [10, 11],
                        15: [12, 13, 14]}
            # HAM warmup: ~6us of dependency-free matmuls on a scratch tile
            # so the clock gate reaches 8/8 while the first input DMAs land
            # (otherwise the first ~20us of projections run at 1.2 GHz).
            wup = const.tile([P, 512], MM_DT, tag="wup")
            nc.vector.memset(wup[:], 0.25)
            for _ in range(14):
                wps = pp.tile([P, 512], F32, tag="pp", name="wps")
                nc.tensor.matmul(wps[:], wup[:, 0:P], wup[:],
                                 start=True, stop=True)
            for u in proj_units(0):
                u()
            va_ones()
            for tt in range(1, 4):
                for u in proj_units(tt):
                    u()
            for g in range(NT):
                b, qt = divmod(g, 4)
                fillers = []
                for t in PROJ_AT.get(g, []):
                    fillers += proj_units(t)
                for t in OPROJ_AT.get(g, []):
                    fillers += oproj_units(t)
                attention(b, qt, fillers)
            for u in oproj_units(NT - 1, scalar_cast=True):
                u()

    nc.compile()
    return nc


def _get_nc():
    global _NC
    if _NC is None:
        _NC = _build()
    return _NC


def _to_tiled_T(x2):
    """[SQ, D] fp32 -> [NT, 128, KO, 512] bf16 with x[g, d] at
    [g//512, d%128, d//128, g%512]."""
    xh = x2.astype(_np_mm_dt())
    return np.ascontiguousarray(
        xh.reshape(NT, 512, KO, P).transpose(0, 3, 2, 1))


def _weight_T(w_slice):
    """[128 out, 1024 in] -> [128 p, KO, 128 m] bf16 with W[m, d] at
    [d%128, d//128, m]."""
    return np.ascontiguousarray(
        w_slice.T.reshape(KO, P, P).transpose(1, 0, 2)).astype(_np_mm_dt())


def kernel(q, k, v, mask, W_q, W_k, W_v, W_o, b_o):
    global LAST_RESULT
    nc = _get_nc()

    qT = _to_tiled_T(np.asarray(q, np.float32).reshape(SQ, D))
    kT = _to_tiled_T(np.asarray(k, np.float32).reshape(SQ, D))
    vT = _to_tiled_T(np.asarray(v, np.float32).reshape(SQ, D))

    p_idx = np.arange(P)[:, None]
    f_idx = np.arange(P)[None, :]
    U = (f_idx < p_idx).astype(np.float32)           # masked: q < k
    Dneg = -384.0 * np.eye(P, dtype=np.float32)
    masks = np.stack([U, Dneg], axis=1).astype(_np_mm_dt())  # [P, 2, P]

    W_q = np.asarray(W_q, np.float32)
    W_k = np.asarray(W_k, np.float32)
    W_v = np.asarray(W_v, np.float32)
    W_o = np.asarray(W_o, np.float32)

    in_maps = []
    for c in range(NCORES):
        cs = slice(c * P, (c + 1) * P)
        in_maps.append({
            "qT": qT, "kT": kT, "vT": vT, "masks": masks,
            "wq": _weight_T(W_q[cs, :]),
            "wk": _weight_T(W_k[cs, :]),
            "wv": _weight_T(W_v[cs, :]),
            # [k, mo, m] = W_o[mo*128+m, c*128+k]
            "wo": np.ascontiguousarray(
                W_o[:, cs].reshape(KO, P, P).transpose(2, 0, 1)
            ).astype(_np_mm_dt()),
        })

    res = run_bass_kernel_spmd(nc, in_maps, core_ids=list(range(NCORES)),
                               trace=TRACE)
    LAST_RESULT = res

    acc = np.zeros((SQ, D), np.float32)
    for c in range(NCORES):
        partial_T = res.results[c]["out"].reshape(D, SQ)
        acc += partial_T.T.astype(np.float32)
    acc += np.asarray(b_o, np.float32)
    return acc.reshape(B, S, D)


# revision 28
# speedup vs baseline: 1.0731x; 1.0080x over previous
"""Multi-head attention (B=4, S=2048, D=1024, H=16) on 8 Trainium2 NeuronCores.

Sharding: tensor-parallel over heads. Core c owns heads 2c, 2c+1 (a 128-wide
slice of the model dim). Each core computes Q/K/V projections for its head
slice over all tokens, causal attention for its 2 heads, and a partial output
projection (contraction over its 128 x-dims). The host sums the 8 partial
outputs and adds b_o.

All matmuls run in bf16 (full PE rate) with fp32 PSUM accumulation; softmax
runs without max-subtraction (scores are O(10), exp stays in range).

On-device layouts (T = transposed, tokens on the free axis):
  QT/KT: [128 head-dims, 8192 tokens] bf16 in SBUF
  VA:    [128 token-chunk, 64 chunks, 256] bf16; cols 0-127 = V dims,
         cols 128-255 = ones (gives replicated softmax row-sums for free)
  Scores are computed transposed, S.T = [k-tokens, q-tokens], so softmax
  normalization lands on the free axis after the attn@V matmul.

Scheduling (keeps the PE dense so the HAM clock gate stays at 2.4 GHz):
  - The scores/exp pipeline runs two chunks ahead of the attn@V matmuls so
    the ScalarE exp (and the GpSimd mask multiply on diagonal chunks) is
    always covered by PE work.
  - Q/K/V projection and output-projection matmuls are chopped into
    single-matmul "filler" units and interleaved between attention chunks,
    so exp-wait bubbles are filled and there is no serialized output-
    projection tail at the end of the kernel.

"""

import sys
import types

sys.path.insert(0, "/opt/trn_rl_repo")

import numpy as np

# Optional: make run_bass_kernel_spmd(trace=True) work on images whose antenv
# lacks axon_hooks. Harmless if unavailable; kernel() defaults to trace=False.
try:  # pragma: no cover
    import antenv
    if "antenv.axon_hooks" not in sys.modules:
        from trn_agent_boot.trn_boot import _ntff_profile_via_ctypes

        _hook = _ntff_profile_via_ctypes("/opt/axon/libaxon_pjrt.so")
        _mod = types.ModuleType("antenv.axon_hooks")
        _mod.get_axon_ntff_profile_hook = lambda: _hook
        _mod.set_axon_ntff_profile_hook = lambda h: None
        sys.modules["antenv.axon_hooks"] = _mod
        antenv.axon_hooks = _mod
except Exception:
    pass

import concourse.bass as bass
import concourse.bacc as bacc
import concourse.tile as tile
import concourse.mybir as mybir
from concourse.bass_utils import run_bass_kernel_spmd

B, S, D, H = 4, 2048, 1024, 16
DK = D // H          # 64
P = 128
SQ = B * S           # 8192 tokens
NT = SQ // 512       # 16 token tiles of 512
KO = D // P          # 8 contraction chunks
NCORES = 8
F16 = mybir.dt.float16
F32 = mybir.dt.float32

TRACE = False        # set by test.py to capture an NTFF profile
LAST_RESULT = None   # BassKernelResults of the most recent run

MM_DT = mybir.dt.bfloat16

_NC = None


def _np_mm_dt():
    if MM_DT == mybir.dt.float16:
        return np.float16
    import ml_dtypes
    return ml_dtypes.bfloat16


def _build():
    nc = bacc.Bacc("TRN2", target_bir_lowering=False, debug=False,
                   num_devices=NCORES)

    qT_d = nc.dram_tensor("qT", [NT, P, KO, 512], MM_DT, kind="ExternalInput")
    kT_d = nc.dram_tensor("kT", [NT, P, KO, 512], MM_DT, kind="ExternalInput")
    vT_d = nc.dram_tensor("vT", [NT, P, KO, 512], MM_DT, kind="ExternalInput")
    wq_d = nc.dram_tensor("wq", [P, KO, P], MM_DT, kind="ExternalInput")
    wk_d = nc.dram_tensor("wk", [P, KO, P], MM_DT, kind="ExternalInput")
    wv_d = nc.dram_tensor("wv", [P, KO, P], MM_DT, kind="ExternalInput")
    wo_d = nc.dram_tensor("wo", [P, KO, P], MM_DT, kind="ExternalInput")
    # masks[:, 0, :] = U (strictly-lower-triangular ones, masked positions)
    # masks[:, 1, :] = -384 * I  (bias matmul: masked logits -> -48 post-scale)
    mk_d = nc.dram_tensor("masks", [P, 2, P], MM_DT, kind="ExternalInput")
    out_d = nc.dram_tensor("out", [KO, P, NT, 512], F16, kind="ExternalOutput")

    with tile.TileContext(nc) as tc:
        with (
            tc.tile_pool(name="const", bufs=1) as const,
            tc.tile_pool(name="persist", bufs=1) as persist,
            tc.tile_pool(name="stream", bufs=3) as stream,
            tc.tile_pool(name="epool", bufs=4) as epool,
            tc.tile_pool(name="rpool", bufs=4) as rpool,
            tc.tile_pool(name="ostage", bufs=4) as ostage,
            tc.tile_pool(name="vstage", bufs=2) as vstage,
            tc.tile_pool(name="pp", bufs=2, space="PSUM") as pp,
            tc.tile_pool(name="scp", bufs=2, space="PSUM") as scp,
            tc.tile_pool(name="opp", bufs=2, space="PSUM") as opp,
        ):
            wq_t = const.tile([P, KO, P], MM_DT, tag="wq")
            wk_t = const.tile([P, KO, P], MM_DT, tag="wk")
            wv_t = const.tile([P, KO, P], MM_DT, tag="wv")
            wo_t = const.tile([P, KO, P], MM_DT, tag="wo")
            mk_t = const.tile([P, 2, P], MM_DT, tag="mk")
            nc.sync.dma_start(wq_t[:], wq_d.ap())
            nc.sync.dma_start(wk_t[:], wk_d.ap())
            nc.sync.dma_start(wv_t[:], wv_d.ap())
            nc.sync.dma_start(wo_t[:], wo_d.ap())
            nc.sync.dma_start(mk_t[:], mk_d.ap())

            ident = const.tile([P, P], MM_DT, tag="ident")
            from concourse.masks import make_identity
            make_identity(nc, ident[:])

            QT = persist.tile([P, SQ], MM_DT, tag="QT")
            KT = persist.tile([P, SQ], MM_DT, tag="KT")
            VA = persist.tile([P, SQ // P, 256], MM_DT, tag="VA")
            XT = persist.tile([P, SQ], MM_DT, tag="XT")

            def va_ones():
                # ones columns for the row-sum trick; per-chunk layout is
                # [A dims 0:64 | ones 64:128 | B dims 128:192 | ones 192:256]
                # On GpSimd: the DVE is busy evacuating the first projection
                # tiles, and a 7us DVE memset there stalls the PE on PSUM WAR.
                nc.gpsimd.memset(VA[:, :, DK:P], 1.0)
                nc.gpsimd.memset(VA[:, :, P + DK:2 * P], 1.0)

            def proj_units(tt):
                """Q/K/V projections for token tile tt as 28 single-matmul
                filler units. DMA-in starts immediately."""
                cols = bass.ts(tt, 512)
                qin = stream.tile([P, KO, 512], MM_DT, tag="qin")
                nc.sync.dma_start(qin[:], qT_d.ap()[tt])
                kin = stream.tile([P, KO, 512], MM_DT, tag="kin")
                nc.sync.dma_start(kin[:], kT_d.ap()[tt])
                vin = stream.tile([P, KO, 512], MM_DT, tag="vin")
                nc.sync.dma_start(vin[:], vT_d.ap()[tt])

                box = {}

                def mk_mm(w_t, xin, key, ko, final):
                    def u():
                        if ko == 0:
                            box[key] = pp.tile([P, 512], F32, tag="pp",
                                               name="ps_" + key)
                        nc.tensor.matmul(box[key][:], w_t[:, ko, :],
                                         xin[:, ko, :],
                                         start=(ko == 0), stop=(ko == KO - 1))
                        if ko == KO - 1:
                            final(box[key])
                    return u

                def qfin(ps):
                    nc.vector.tensor_copy(QT[:, cols], ps[:])

                def kfin(ps):
                    nc.vector.tensor_copy(KT[:, cols], ps[:])

                def vfin(ps):
                    vts = vstage.tile([P, 512], MM_DT, tag="vts", name="vts")
                    nc.vector.tensor_copy(vts[:], ps[:])
                    box["vts"] = vts

                def mk_tp(sub):
                    def u():
                        tp = pp.tile([P, P], MM_DT, tag="pp", name="tp")
                        nc.tensor.transpose(tp[:],
                                            box["vts"][:, bass.ts(sub, P)],
                                            ident[:])
                        kc = tt * 4 + sub
                        # one strided copy: head halves land at cols 0:64,
                        # 128:192
                        nc.vector.tensor_copy(
                            VA[:, kc].rearrange("p (a x) -> p a x",
                                                a=2)[:, :, 0:DK],
                            tp[:].rearrange("p (a x) -> p a x", a=2))
                    return u

                units = []
                for ko in range(KO):
                    units.append(mk_mm(wq_t, qin, "q", ko, qfin))
                for ko in range(KO):
                    units.append(mk_mm(wk_t, kin, "k", ko, kfin))
                for ko in range(KO):
                    units.append(mk_mm(wv_t, vin, "v", ko, vfin))
                for sub in range(4):
                    units.append(mk_tp(sub))
                return units

            def oproj_units(tt, scalar_cast=False):
                """Output projection for token tile tt: 8 filler units, one
                per 128-wide output-dim chunk (matmul + cast + DMA-out)."""
                def mk(mo):
                    def u():
                        pso = pp.tile([P, 512], F32, tag="pp", name="pso")
                        nc.tensor.matmul(pso[:], wo_t[:, mo, :],
                                         XT[:, bass.ts(tt, 512)],
                                         start=True, stop=True)
                        ost = ostage.tile([P, 512], F16, tag="ost",
                                          name="ost")
                        if scalar_cast and (mo % 2 == 1):
                            nc.scalar.activation(
                                ost[:], pso[:],
                                mybir.ActivationFunctionType.Copy)
                        else:
                            nc.vector.tensor_copy(ost[:], pso[:])
                        nc.sync.dma_start(out_d.ap()[mo, :, tt, :], ost[:])
                    return u
                return [mk(mo) for mo in range(KO)]

            def attention(b, qt, fillers):
                """One 512-query tile of causal attention, both heads.

                scores+exp run two k-chunks ahead of the attn@V matmuls, in
                groups of two chunks: the 4 row-tiled score matmuls of a
                group are emitted back-to-back (entering/leaving row-tiled
                mode costs ~100-190ns, so batch the transitions), then the
                full-array work (mask-bias matmuls, fillers, attn@V).
                Filler units keep the PE dense while ScalarE works through
                the exps.
                """
                qcols = bass.ds(b * S + qt * 512, 512)
                nkc = 4 * qt + 4
                ops = [opp.tile([P, 512], F32, tag="op", name=f"op{h}")
                       for h in range(2)]
                s_box = {}
                e_box = {}

                def sc_mm(kc):
                    kcols = bass.ds(b * S + kc * P, P)
                    j = kc - 4 * qt
                    co = max(j, 0) * P  # valid q-columns start here (causal)
                    w = 512 - co
                    # both heads' scores live in one double-bank tile so a
                    # single ScalarE exp instruction covers the pair (the
                    # ~200ns fixed cost per activation dominates Scalar's
                    # backlog otherwise)
                    ssc = scp.tile([P, 2, 512], F32, tag="sc", name="ssc")
                    for h in range(2):
                        rb = h * DK
                        nc.tensor.matmul(
                            ssc[:, h, co:],
                            KT[rb:rb + DK, kcols],
                            QT[rb:rb + DK, bass.ds(b * S + qt * 512 + co, w)],
                            start=True, stop=(j < 0))
                    s_box[kc] = (ssc, co, j >= 0)

                def mask_exp(kc):
                    ssc, co, diag = s_box.pop(kc)
                    if diag:
                        # accumulate -384 * U into the diagonal block:
                        # masked (q < k) logits drop to ~-48 post-scale
                        for h in range(2):
                            nc.tensor.matmul(
                                ssc[:, h, co:co + P],
                                mk_t[:, 1, :], mk_t[:, 0, :],
                                start=False, stop=True)
                    e_t = epool.tile([P, 2, 512], MM_DT, tag="e", name="e_t")
                    nc.scalar.activation(e_t[:, :, co:], ssc[:, :, co:],
                                         mybir.ActivationFunctionType.Exp,
                                         scale=0.125)
                    e_box[kc] = (e_t, co)

                def attn_mm(kc):
                    gkc = b * (S // P) + kc
                    e_t, co = e_box.pop(kc)
                    for h in range(2):
                        nc.tensor.matmul(ops[h][:, co:],
                                         VA[:, gkc, bass.ts(h, P)],
                                         e_t[:, h, co:],
                                         start=(kc == 0), stop=(kc == nkc - 1))

                def pop_fillers(kc):
                    left = nkc - kc
                    n = (len(fillers) * 2 + left - 1) // left
                    for _ in range(n):
                        if fillers:
                            fillers.pop(0)()

                sc_mm(0)
                sc_mm(1)
                mask_exp(0)
                mask_exp(1)
                for kc in range(0, nkc, 2):
                    if kc + 2 < nkc:
                        sc_mm(kc + 2)
                        sc_mm(kc + 3)
                        mask_exp(kc + 2)
                        mask_exp(kc + 3)
                    pop_fillers(kc)
                    attn_mm(kc)
                    attn_mm(kc + 1)
                while fillers:
                    fillers.pop(0)()

                for h in range(2):
                    r_t = rpool.tile([DK, 512], F32, tag="r", name="r_t")
                    s_t = rpool.tile([DK, 512], F32, tag="s", name="s_t")
                    nc.vector.tensor_copy(s_t[:], ops[h][DK:P, :])
                    nc.vector.reciprocal_approx_fast(r_t[:], s_t[:])
                    nc.vector.tensor_mul(XT[h * DK:(h + 1) * DK, qcols],
                                         ops[h][0:DK, :], r_t[:])

            # Batch-0 projections up front (they cover the DMA-in latency and
            # warm the HAM clock gate); everything else is interleaved into
            # the attention stream as single-matmul fillers. The assignment
            # pushes projection sets as late as their deadlines allow and
            # reserves output-projection sets for the late tiles, so the
            # exp-paced qt=3 tiles (especially batch 3, which has no
            # projection work left) still keep the PE dense and the HAM
            # clock gate at 8/8.
            PROJ_AT = {0: [4, 5], 1: [6, 7, 8], 2: [9], 3: [10],
                       6: [11], 7: [12], 10: [13], 11: [14], 14: [15]}
            OPROJ_AT = {3: [0], 4: [1], 5: [2, 3], 7: [4], 8: [5],
                        9: [6, 7], 11: [8], 12: [9], 13: [10, 11],
                        15: [12, 13, 14]}
            # HAM warmup: ~6us of dependency-free matmuls on a scratch tile
            # so the clock gate reaches 8/8 while the first input DMAs land
            # (otherwise the first ~20us of projections run at 1.2 GHz).
            wup = const.tile([P, 512], MM_DT, tag="wup")
            nc.vector.memset(wup[:], 0.25)
            for _ in range(14):
                wps = pp.tile([P, 512], F32, tag="pp", name="wps")
                nc.tensor.matmul(wps[:], wup[:, 0:P], wup[:],
                                 start=True, stop=True)
            for u in proj_units(0):
                u()
            va_ones()
            for tt in range(1, 4):
                for u in proj_units(tt):
                    u()
            for g in range(NT):
                b, qt = divmod(g, 4)
                fillers = []
                for t in PROJ_AT.get(g, []):
                    fillers += proj_units(t)
                for t in OPROJ_AT.get(g, []):
                    # late tiles: ScalarE is winding down, split the PSUM
                    # evacuation casts between DVE and ScalarE
                    fillers += oproj_units(t, scalar_cast=(g >= 13))
                attention(b, qt, fillers)
            for u in oproj_units(NT - 1, scalar_cast=True):
                u()

    nc.compile()
    return nc


def _get_nc():
    global _NC
    if _NC is None:
        _NC = _build()
    return _NC


def _to_tiled_T(x2):
    """[SQ, D] fp32 -> [NT, 128, KO, 512] bf16 with x[g, d] at
    [g//512, d%128, d//128, g%512]."""
    xh = x2.astype(_np_mm_dt())
    return np.ascontiguousarray(
        xh.reshape(NT, 512, KO, P).transpose(0, 3, 2, 1))


def _weight_T(w_slice):
    """[128 out, 1024 in] -> [128 p, KO, 128 m] bf16 with W[m, d] at
    [d%128, d//128, m]."""
    return np.ascontiguousarray(
        w_slice.T.reshape(KO, P, P).transpose(1, 0, 2)).astype(_np_mm_dt())


def kernel(q, k, v, mask, W_q, W_k, W_v, W_o, b_o):
    global LAST_RESULT
    nc = _get_nc()

    qT = _to_tiled_T(np.asarray(q, np.float32).reshape(SQ, D))
    kT = _to_tiled_T(np.asarray(k, np.float32).reshape(SQ, D))
    vT = _to_tiled_T(np.asarray(v, np.float32).reshape(SQ, D))

    p_idx = np.arange(P)[:, None]
    f_idx = np.arange(P)[None, :]
    U = (f_idx < p_idx).astype(np.float32)           # masked: q < k
    Dneg = -384.0 * np.eye(P, dtype=np.float32)
    masks = np.stack([U, Dneg], axis=1).astype(_np_mm_dt())  # [P, 2, P]

    W_q = np.asarray(W_q, np.float32)
    W_k = np.asarray(W_k, np.float32)
    W_v = np.asarray(W_v, np.float32)
    W_o = np.asarray(W_o, np.float32)

    in_maps = []
    for c in range(NCORES):
        cs = slice(c * P, (c + 1) * P)
        in_maps.append({
            "qT": qT, "kT": kT, "vT": vT, "masks": masks,
            "wq": _weight_T(W_q[cs, :]),
            "wk": _weight_T(W_k[cs, :]),
            "wv": _weight_T(W_v[cs, :]),
            # [k, mo, m] = W_o[mo*128+m, c*128+k]
            "wo": np.ascontiguousarray(
                W_o[:, cs].reshape(KO, P, P).transpose(2, 0, 1)
            ).astype(_np_mm_dt()),
        })

    res = run_bass_kernel_spmd(nc, in_maps, core_ids=list(range(NCORES)),
                               trace=TRACE)
    LAST_RESULT = res

    acc = np.zeros((SQ, D), np.float32)
    for c in range(NCORES):
        partial_T = res.results[c]["out"].reshape(D, SQ)
        acc += partial_T.T.astype(np.float32)
    acc += np.asarray(b_o, np.float32)
    return acc.reshape(B, S, D)
